# revision 38
# baseline (speedup 1.0000x reference)
"""Trainium2 Bass kernel for nn_DiscreteAttention_48335561949594.

Discrete-attention EM loop:
    p = softmax(inputs, -1)                      (loop-invariant)
    5x: log_q = log_softmax(theta); logits = p @ log_q^T + log(pi+eps)
        gamma = softmax_k(logits); pi = gamma.mean(n)
        theta = (gamma^T @ inputs) / (gamma.sum(n) + eps)
    out = softmax(theta + gumbel)

Sharding: pure data-parallel over B (B == 8 == n_cores), one batch per
NeuronCore; slot tensors replicated. Zero collectives.

Per-core layout (SBUF-resident across all 5 iterations, all bf16 so every
matmul runs single-pass with FWL weight loads — fp32 matmuls on trn2 run
2-pass LOW_HIGH and disable FWL):
    xones [128, 128, 257] bf16  x tiles n-major (+ ones column for gamma_sum)
    pT    [128, 2, 16384] bf16  p transposed d-major (matmul weights)
x is cast f32->bf16 inside the (gpsimd) DMA; pT is produced by DMA-transpose.
Each EM iteration runs entirely from SBUF:
    logits tile [128n,16k] = sum_c pT_chunk^T @ logqT_chunk   (PE, bf16 FWL)
    gamma = exp(logits) * recip(rowsum)                       (ACT + DVE)
    numerator[16k, 257] += gamma^T @ [x | 1]                  (PE, f32 psum acc)
log(pi+eps) is folded into log_q (sum_d p = 1), so the logits bias is free.
"""

import numpy as np

NUM_ITER = 5
K = 16          # slots
D = 256         # input size
EPS = 1e-8
B = 8
N = 16384
P = 128         # partitions
NT = N // P     # 128 n-tiles per core
G = 16          # n-tiles per gamma group
NG = NT // G    # 8 groups

_CACHE = {}


def _build_nc():
    import concourse.bass as bass
    import concourse.tile as tile
    from concourse import bacc, mybir
    from concourse.masks import make_identity

    f32 = mybir.dt.float32
    bf16 = mybir.dt.bfloat16
    AF = mybir.ActivationFunctionType
    AX = mybir.AxisListType

    nc = bacc.Bacc("TRN2", target_bir_lowering=False, debug=False)
    x = nc.dram_tensor("x", (N, D), f32, kind="ExternalInput").ap()
    theta0 = nc.dram_tensor("theta0", (K, D), f32, kind="ExternalInput").ap()
    pi0 = nc.dram_tensor("pi0", (K, 1), f32, kind="ExternalInput").ap()
    g_in = nc.dram_tensor("g", (K, D), f32, kind="ExternalInput").ap()
    out = nc.dram_tensor("out", (K, D), f32, kind="ExternalOutput").ap()

    with (
        tile.TileContext(nc) as tc,
        tc.tile_pool(name="persist", bufs=1) as persist,
        tc.tile_pool(name="small", bufs=2) as small,
        tc.tile_pool(name="work", bufs=4) as work,
        tc.tile_pool(name="once", bufs=1) as once,
        tc.tile_pool(name="stats", bufs=4) as stats,
        tc.tile_pool(name="gwork", bufs=3) as gwork,
        tc.tile_pool(name="pnum", bufs=2, space="PSUM") as pnum,
        tc.tile_pool(name="plog", bufs=3, space="PSUM") as plog,
        tc.tile_pool(name="ptr", bufs=2, space="PSUM") as ptr,
        tc.tile_pool(name="dram", bufs=1, space="DRAM") as dram,
    ):
        xones = persist.tile([P, NT, D + 1], bf16, tag="xones")
        pT = persist.tile([P, 2, N], bf16, tag="pT")
        nc.vector.memset(xones[:, :, D : D + 1], 1.0)
        identb = persist.tile([P, P], bf16, tag="identb")
        make_identity(nc, identb[:, :])

        # ---- setup: load x (cast to bf16 in-DMA), p = softmax(x), pT ----
        p_dram = dram.tile([N, D], bf16, tag="p_dram")
        pdr_r = p_dram[:, :].rearrange("(t p) d -> p t d", p=P)
        xr = x.rearrange("(t p) d -> p t d", p=P)
        DCH = 8  # tiles per casting DMA
        for jc in range(NT // DCH):
            nc.gpsimd.dma_start(
                out=xones[:, jc * DCH : (jc + 1) * DCH, 0:D],
                in_=xr[:, jc * DCH : (jc + 1) * DCH, :],
            )
        SB = 4  # setup batch: tiles per ACT exp / DVE reduce op
        for jb in range(NT // SB):
            j0 = jb * SB
            e_t = work.tile([P, SB, D], bf16, tag="e_t")
            # softmax without max-subtraction: x ~ N(0,1), exp safe in f32
            nc.scalar.activation(e_t[:, :, :], xones[:, j0 : j0 + SB, 0:D], AF.Exp)
            s_t = stats.tile([P, SB], f32, tag="s_t")
            nc.vector.reduce_sum(s_t[:, :], e_t[:, :, :], axis=AX.X)
            r_t = stats.tile([P, SB], f32, tag="r_t")
            nc.vector.reciprocal(r_t[:, :], s_t[:, :])
            p_t = work.tile([P, SB, D], bf16, tag="p_t")
            r_bcast = bass.AP(
                tensor=r_t.tensor,
                offset=r_t.offset,
                ap=[r_t.ap[0], r_t.ap[1], [0, D]],
            )
            nc.vector.tensor_mul(p_t[:, :, :], e_t[:, :, :], r_bcast)
            # bounce p via DRAM so the d-major pT can be built with a few
            # big DMA transposes instead of 256 PE transposes + PSUM copies
            nc.gpsimd.dma_start(
                out=pdr_r[:, jb * SB : (jb + 1) * SB, :], in_=p_t[:, :, :]
            )
            TCH = 2048 // P  # tiles per DMA-transpose chunk (2048 n rows)
            if (jb + 1) * SB % TCH == 0:
                n0 = ((jb + 1) * SB - TCH) * P
                for c in range(2):
                    nc.sync.dma_start(
                        out=pT[:, c, n0 : n0 + TCH * P],
                        in_=p_dram[n0 : n0 + TCH * P, c * P : (c + 1) * P],
                        transpose=True,
                    )

        # ---- initial theta / pi' = pi + eps (host adds eps for iter 0) ----
        theta_sb = small.tile([K, D], f32, tag="theta")
        nc.sync.dma_start(out=theta_sb[:, :], in_=theta0)
        pi_sb = small.tile([K, 1], f32, tag="pi")
        nc.sync.dma_start(out=pi_sb[:, :], in_=pi0)

        for it in range(NUM_ITER):
            # Instead of log-space: lq = theta - max, and gamma weights get a
            # multiplicative w_k = (pi_k+eps)/s_k with s = sum_d exp(theta-max).
            # Avoids every Ln (no ACT table swaps; Exp stays loaded all kernel).
            mneg = stats.tile([K, 1], f32, tag="mneg")
            nc.vector.reduce_max(mneg[:, :], theta_sb[:, :], axis=AX.X, negate=True)
            eth = once.tile([K, D], f32, tag="kscratch")
            ssum = stats.tile([K, 1], f32, tag="ssum")
            nc.scalar.activation(
                eth[:, :], theta_sb[:, :], AF.Exp, bias=mneg[:, :], accum_out=ssum[:, :]
            )
            rs = stats.tile([K, 1], f32, tag="rs")
            nc.vector.reciprocal(rs[:, :], ssum[:, :])
            w_sb = stats.tile([K, 1], bf16, tag="w_sb")
            nc.vector.tensor_mul(w_sb[:, :], rs[:, :], pi_sb[:, :])
            lq = once.tile([K, D], bf16, tag="lq")
            nc.vector.tensor_scalar_add(lq[:, :], theta_sb[:, :], mneg[:, :])
            lqT = small.tile([P, 2, K], bf16, tag="lqT")
            for c in range(2):
                plq_t = ptr.tile([P, K], bf16, tag="tr")
                nc.tensor.transpose(
                    plq_t[:, :], lq[:, c * P : (c + 1) * P], identb[:K, :K]
                )
                nc.vector.tensor_copy(lqT[:, c, :], plq_t[:, :])
            # broadcast w over partitions: w_rep[p,k] = sum_c w[c]*I[c,k]
            w_bc = bass.AP(
                tensor=w_sb.tensor, offset=w_sb.offset, ap=[w_sb.ap[0], [0, P]]
            )
            wr_ps = ptr.tile([P, K], f32, tag="tr")
            nc.tensor.matmul(wr_ps[:, :], w_bc, identb[:K, :K], start=True, stop=True)
            w_rep = stats.tile([P, K], f32, tag="w_rep")
            nc.vector.tensor_copy(w_rep[:, :], wr_ps[:, :])

            num_ps = pnum.tile([K, D + 1], f32)
            for g in range(NG):
                lg_ps = plog.tile([P, G, K], f32)
                for t in range(G):
                    j = g * G + t
                    for c in range(2):
                        nc.tensor.matmul(
                            lg_ps[:, t, :],
                            pT[:, c, j * P : (j + 1) * P],
                            lqT[:, c, :],
                            start=(c == 0),
                            stop=(c == 1),
                        )
                eg = gwork.tile([P, G, K], bf16, tag="eg")
                nc.scalar.activation(eg[:, :, :], lg_ps[:, :, :], AF.Exp)
                e2 = gwork.tile([P, G, K], bf16, tag="e2")
                w_bcast = bass.AP(
                    tensor=w_rep.tensor,
                    offset=w_rep.offset,
                    ap=[w_rep.ap[0], [0, G], w_rep.ap[1]],
                )
                nc.vector.tensor_mul(e2[:, :, :], eg[:, :, :], w_bcast)
                sg = stats.tile([P, G], f32, tag="sg")
                nc.vector.reduce_sum(sg[:, :], e2[:, :, :], axis=AX.X)
                rg = stats.tile([P, G], f32, tag="rg")
                nc.vector.reciprocal(rg[:, :], sg[:, :])
                gm = gwork.tile([P, G, K], bf16, tag="gm")
                rg_bcast = bass.AP(
                    tensor=rg.tensor,
                    offset=rg.offset,
                    ap=[rg.ap[0], rg.ap[1], [0, K]],
                )
                nc.vector.tensor_mul(gm[:, :, :], e2[:, :, :], rg_bcast)
                for t in range(G):
                    j = g * G + t
                    nc.tensor.matmul(
                        num_ps[:, :],
                        gm[:, t, :],
                        xones[:, j, :],
                        start=(j == 0),
                        stop=(j == NT - 1),
                    )

            gs = stats.tile([K, 1], f32, tag="gs")
            nc.vector.tensor_scalar_add(gs[:, :], num_ps[:, D : D + 1], EPS)
            rgs = stats.tile([K, 1], f32, tag="rgs")
            nc.vector.reciprocal(rgs[:, :], gs[:, :])
            theta_sb = small.tile([K, D], f32, tag="theta")
            nc.vector.tensor_scalar_mul(theta_sb[:, :], num_ps[:, 0:D], rgs[:, :])
            pi_sb = small.tile([K, 1], f32, tag="pi")
            nc.vector.tensor_scalar(
                pi_sb[:, :], num_ps[:, D : D + 1], 1.0 / N, EPS,
                op0=mybir.AluOpType.mult, op1=mybir.AluOpType.add,
            )

        # ---- out = softmax(theta + gumbel) ----
        gsb = once.tile([K, D], f32, tag="gsb")
        nc.sync.dma_start(out=gsb[:, :], in_=g_in)
        nc.vector.tensor_add(gsb[:, :], theta_sb[:, :], gsb[:, :])
        mo = stats.tile([K, 1], f32, tag="mo")
        nc.vector.reduce_max(mo[:, :], gsb[:, :], axis=AX.X, negate=True)
        eo = once.tile([K, D], f32, tag="kscratch")
        so = stats.tile([K, 1], f32, tag="so")
        nc.scalar.activation(eo[:, :], gsb[:, :], AF.Exp, bias=mo[:, :], accum_out=so[:, :])
        ro = stats.tile([K, 1], f32, tag="ro")
        nc.vector.reciprocal(ro[:, :], so[:, :])
        nc.vector.tensor_scalar_mul(eo[:, :], eo[:, :], ro[:, :])
        nc.sync.dma_start(out=out, in_=eo[:, :])

    nc.compile()
    return nc


def _gumbel_host():
    import jax

    with jax.default_device(jax.devices("cpu")[0]):
        g = jax.random.gumbel(jax.random.key(1), (B, K, D), dtype="float32")
        return np.asarray(g)


def kernel(inputs, slot_logits, mixing_coefficients):
    from concourse import bass_utils

    inputs = np.ascontiguousarray(np.asarray(inputs, dtype=np.float32))
    theta0 = np.ascontiguousarray(np.asarray(slot_logits, dtype=np.float32)[0])
    # device tracks pi' = pi + eps (the only form the math needs)
    pi0 = np.ascontiguousarray(
        np.asarray(mixing_coefficients, dtype=np.float32).reshape(K, 1)
        + np.float32(EPS)
    )
    assert inputs.shape == (B, N, D)

    if "nc" not in _CACHE:
        _CACHE["nc"] = _build_nc()
        _CACHE["g"] = _gumbel_host()
    nc = _CACHE["nc"]
    g = _CACHE["g"]

    in_maps = [
        {
            "x": inputs[b],
            "theta0": theta0,
            "pi0": pi0,
            "g": np.ascontiguousarray(g[b]),
        }
        for b in range(B)
    ]
    res = bass_utils.run_bass_kernel_spmd(nc, in_maps, core_ids=list(range(B)))
    return np.stack([res.results[b]["out"] for b in range(B)], axis=0)


# revision 45
# speedup vs baseline: 1.2081x; 1.2081x over previous
"""Trainium2 Bass kernel for nn_DiscreteAttention_48335561949594.

Discrete-attention EM loop:
    p = softmax(inputs, -1)                      (loop-invariant)
    5x: log_q = log_softmax(theta); logits = p @ log_q^T + log(pi+eps)
        gamma = softmax_k(logits); pi = gamma.mean(n)
        theta = (gamma^T @ inputs) / (gamma.sum(n) + eps)
    out = softmax(theta + gumbel)

Sharding: pure data-parallel over B (B == 8 == n_cores), one batch per
NeuronCore; slot tensors replicated. Zero collectives.

Per-core layout (SBUF-resident across all 5 iterations, all bf16 so every
matmul runs single-pass with FWL weight loads — fp32 matmuls on trn2 run
2-pass LOW_HIGH and disable FWL):
    xones [128, 128, 257] bf16  x tiles n-major (+ ones column for gamma_sum)
    pT    [128, 2, 16384] bf16  p transposed d-major (matmul weights)
x is cast f32->bf16 inside the (gpsimd) DMA; pT is produced by DMA-transpose.
Each EM iteration runs entirely from SBUF:
    logits tile [128n,16k] = sum_c pT_chunk^T @ logqT_chunk   (PE, bf16 FWL)
    gamma = exp(logits) * recip(rowsum)                       (ACT + DVE)
    numerator[16k, 257] += gamma^T @ [x | 1]                  (PE, f32 psum acc)
log(pi+eps) is folded into log_q (sum_d p = 1), so the logits bias is free.
"""

import numpy as np

NUM_ITER = 5
K = 16          # slots
D = 256         # input size
EPS = 1e-8
B = 8
N = 16384
P = 128         # partitions
NT = N // P     # 128 n-tiles per core
G = 16          # n-tiles per gamma group
NG = NT // G    # 8 groups

_CACHE = {}


def _build_nc():
    import concourse.bass as bass
    import concourse.tile as tile
    from concourse import bacc, mybir
    from concourse.masks import make_identity

    f32 = mybir.dt.float32
    bf16 = mybir.dt.bfloat16
    AF = mybir.ActivationFunctionType
    AX = mybir.AxisListType

    nc = bacc.Bacc("TRN2", target_bir_lowering=False, debug=False)
    x = nc.dram_tensor("x", (N, D), f32, kind="ExternalInput").ap()
    theta0 = nc.dram_tensor("theta0", (K, D), f32, kind="ExternalInput").ap()
    pi0 = nc.dram_tensor("pi0", (K, 1), f32, kind="ExternalInput").ap()
    g_in = nc.dram_tensor("g", (K, D), f32, kind="ExternalInput").ap()
    out = nc.dram_tensor("out", (K, D), f32, kind="ExternalOutput").ap()

    with (
        tile.TileContext(nc) as tc,
        tc.tile_pool(name="persist", bufs=1) as persist,
        tc.tile_pool(name="small", bufs=2) as small,
        tc.tile_pool(name="work", bufs=4) as work,
        tc.tile_pool(name="once", bufs=1) as once,
        tc.tile_pool(name="stats", bufs=4) as stats,
        tc.tile_pool(name="gwork", bufs=3) as gwork,
        tc.tile_pool(name="pnum", bufs=2, space="PSUM") as pnum,
        tc.tile_pool(name="plog", bufs=3, space="PSUM") as plog,
        tc.tile_pool(name="ptr", bufs=3, space="PSUM") as ptr,
    ):
        xones = persist.tile([P, NT, D + 1], bf16, tag="xones")
        pT = persist.tile([P, 2, N], bf16, tag="pT")
        nc.vector.memset(xones[:, :, D : D + 1], 1.0)
        identb = persist.tile([P, P], bf16, tag="identb")
        make_identity(nc, identb[:, :])
        r_all = persist.tile([P, NT], f32, tag="r_all")

        # ---- setup: load x (cast to bf16 in-DMA), p = softmax(x), pT ----
        xr = x.rearrange("(t p) d -> p t d", p=P)
        DCH = 8  # tiles per casting DMA
        for jc in range(NT // DCH):
            nc.gpsimd.dma_start(
                out=xones[:, jc * DCH : (jc + 1) * DCH, 0:D],
                in_=xr[:, jc * DCH : (jc + 1) * DCH, :],
            )
        SB = 4  # setup batch: tiles per ACT exp / DVE reduce op
        for jb in range(NT // SB):
            j0 = jb * SB
            e_t = work.tile([P, SB, D], bf16, tag="e_t")
            # softmax without max-subtraction: x ~ N(0,1), exp safe in f32
            nc.scalar.activation(e_t[:, :, :], xones[:, j0 : j0 + SB, 0:D], AF.Exp)
            s_t = stats.tile([P, SB], f32, tag="s_t")
            nc.vector.reduce_sum(s_t[:, :], e_t[:, :, :], axis=AX.X)
            # store 1/rowsum; pT stays unnormalized — the scale is applied to
            # the (16x smaller) logits inside each iteration instead
            nc.vector.reciprocal(r_all[:, j0 : j0 + SB], s_t[:, :])
            for t in range(SB):
                j = j0 + t
                ps_t = ptr.tile([P, 2, P], bf16, tag="tr")
                for c in range(2):
                    nc.tensor.transpose(
                        ps_t[:, c, :], e_t[:, t, c * P : (c + 1) * P], identb[:, :]
                    )
                dst = pT[:, :, j * P : (j + 1) * P]
                if j % 5 < 2:
                    nc.scalar.copy(dst, ps_t[:, :, :])
                else:
                    nc.vector.tensor_copy(dst, ps_t[:, :, :])

        # ---- initial theta / pi' = pi + eps (host adds eps for iter 0) ----
        theta_sb = small.tile([K, D], f32, tag="theta")
        nc.sync.dma_start(out=theta_sb[:, :], in_=theta0)
        pi_sb = small.tile([K, 1], f32, tag="pi")
        nc.sync.dma_start(out=pi_sb[:, :], in_=pi0)

        for it in range(NUM_ITER):
            # Instead of log-space: lq = theta - max, and gamma weights get a
            # multiplicative w_k = (pi_k+eps)/s_k with s = sum_d exp(theta-max).
            # Avoids every Ln (no ACT table swaps; Exp stays loaded all kernel).
            mneg = stats.tile([K, 1], f32, tag="mneg")
            nc.vector.reduce_max(mneg[:, :], theta_sb[:, :], axis=AX.X, negate=True)
            eth = once.tile([K, D], f32, tag="kscratch")
            ssum = stats.tile([K, 1], f32, tag="ssum")
            nc.scalar.activation(
                eth[:, :], theta_sb[:, :], AF.Exp, bias=mneg[:, :], accum_out=ssum[:, :]
            )
            rs = stats.tile([K, 1], f32, tag="rs")
            nc.vector.reciprocal(rs[:, :], ssum[:, :])
            w_sb = stats.tile([K, 1], bf16, tag="w_sb")
            nc.vector.tensor_mul(w_sb[:, :], rs[:, :], pi_sb[:, :])
            lq = once.tile([K, D], bf16, tag="lq")
            nc.vector.tensor_scalar_add(lq[:, :], theta_sb[:, :], mneg[:, :])
            lqT = small.tile([P, 2, K], bf16, tag="lqT")
            for c in range(2):
                plq_t = ptr.tile([P, K], bf16, tag="tr")
                nc.tensor.transpose(
                    plq_t[:, :], lq[:, c * P : (c + 1) * P], identb[:K, :K]
                )
                nc.vector.tensor_copy(lqT[:, c, :], plq_t[:, :])
            # broadcast w over partitions: w_rep[p,k] = sum_c w[c]*I[c,k]
            w_bc = bass.AP(
                tensor=w_sb.tensor, offset=w_sb.offset, ap=[w_sb.ap[0], [0, P]]
            )
            wr_ps = ptr.tile([P, K], f32, tag="tr")
            nc.tensor.matmul(wr_ps[:, :], w_bc, identb[:K, :K], start=True, stop=True)
            w_rep = stats.tile([P, K], f32, tag="w_rep")
            nc.vector.tensor_copy(w_rep[:, :], wr_ps[:, :])

            num_ps = pnum.tile([K, D + 1], f32)
            for g in range(NG):
                lg_ps = plog.tile([P, G, K], f32)
                for t in range(G):
                    j = g * G + t
                    for c in range(2):
                        nc.tensor.matmul(
                            lg_ps[:, t, :],
                            pT[:, c, j * P : (j + 1) * P],
                            lqT[:, c, :],
                            start=(c == 0),
                            stop=(c == 1),
                        )
                # apply the deferred softmax normalizer to the raw logits:
                # true_logits = r_n * (e~^T @ lq)
                lsc = gwork.tile([P, G, K], f32, tag="lsc")
                rsl = r_all[:, g * G : (g + 1) * G]
                ra_bcast = bass.AP(
                    tensor=rsl.tensor,
                    offset=rsl.offset,
                    ap=[rsl.ap[0], rsl.ap[1], [0, K]],
                )
                nc.vector.tensor_mul(lsc[:, :, :], lg_ps[:, :, :], ra_bcast)
                eg = gwork.tile([P, G, K], bf16, tag="eg")
                nc.scalar.activation(eg[:, :, :], lsc[:, :, :], AF.Exp)
                e2 = gwork.tile([P, G, K], bf16, tag="e2")
                w_bcast = bass.AP(
                    tensor=w_rep.tensor,
                    offset=w_rep.offset,
                    ap=[w_rep.ap[0], [0, G], w_rep.ap[1]],
                )
                nc.vector.tensor_mul(e2[:, :, :], eg[:, :, :], w_bcast)
                sg = stats.tile([P, G], f32, tag="sg")
                nc.vector.reduce_sum(sg[:, :], e2[:, :, :], axis=AX.X)
                rg = stats.tile([P, G], f32, tag="rg")
                nc.vector.reciprocal(rg[:, :], sg[:, :])
                gm = gwork.tile([P, G, K], bf16, tag="gm")
                rg_bcast = bass.AP(
                    tensor=rg.tensor,
                    offset=rg.offset,
                    ap=[rg.ap[0], rg.ap[1], [0, K]],
                )
                nc.vector.tensor_mul(gm[:, :, :], e2[:, :, :], rg_bcast)
                for t in range(G):
                    j = g * G + t
                    nc.tensor.matmul(
                        num_ps[:, :],
                        gm[:, t, :],
                        xones[:, j, :],
                        start=(j == 0),
                        stop=(j == NT - 1),
                    )

            gs = stats.tile([K, 1], f32, tag="gs")
            nc.vector.tensor_scalar_add(gs[:, :], num_ps[:, D : D + 1], EPS)
            rgs = stats.tile([K, 1], f32, tag="rgs")
            nc.vector.reciprocal(rgs[:, :], gs[:, :])
            theta_sb = small.tile([K, D], f32, tag="theta")
            nc.vector.tensor_scalar_mul(theta_sb[:, :], num_ps[:, 0:D], rgs[:, :])
            pi_sb = small.tile([K, 1], f32, tag="pi")
            nc.vector.tensor_scalar(
                pi_sb[:, :], num_ps[:, D : D + 1], 1.0 / N, EPS,
                op0=mybir.AluOpType.mult, op1=mybir.AluOpType.add,
            )

        # ---- out = softmax(theta + gumbel) ----
        gsb = once.tile([K, D], f32, tag="gsb")
        nc.sync.dma_start(out=gsb[:, :], in_=g_in)
        nc.vector.tensor_add(gsb[:, :], theta_sb[:, :], gsb[:, :])
        mo = stats.tile([K, 1], f32, tag="mo")
        nc.vector.reduce_max(mo[:, :], gsb[:, :], axis=AX.X, negate=True)
        eo = once.tile([K, D], f32, tag="kscratch")
        so = stats.tile([K, 1], f32, tag="so")
        nc.scalar.activation(eo[:, :], gsb[:, :], AF.Exp, bias=mo[:, :], accum_out=so[:, :])
        ro = stats.tile([K, 1], f32, tag="ro")
        nc.vector.reciprocal(ro[:, :], so[:, :])
        nc.vector.tensor_scalar_mul(eo[:, :], eo[:, :], ro[:, :])
        nc.sync.dma_start(out=out, in_=eo[:, :])

    nc.compile()
    return nc


def _gumbel_host():
    import jax

    with jax.default_device(jax.devices("cpu")[0]):
        g = jax.random.gumbel(jax.random.key(1), (B, K, D), dtype="float32")
        return np.asarray(g)


def kernel(inputs, slot_logits, mixing_coefficients):
    from concourse import bass_utils

    inputs = np.ascontiguousarray(np.asarray(inputs, dtype=np.float32))
    theta0 = np.ascontiguousarray(np.asarray(slot_logits, dtype=np.float32)[0])
    # device tracks pi' = pi + eps (the only form the math needs)
    pi0 = np.ascontiguousarray(
        np.asarray(mixing_coefficients, dtype=np.float32).reshape(K, 1)
        + np.float32(EPS)
    )
    assert inputs.shape == (B, N, D)

    if "nc" not in _CACHE:
        _CACHE["nc"] = _build_nc()
        _CACHE["g"] = _gumbel_host()
    nc = _CACHE["nc"]
    g = _CACHE["g"]

    in_maps = [
        {
            "x": inputs[b],
            "theta0": theta0,
            "pi0": pi0,
            "g": np.ascontiguousarray(g[b]),
        }
        for b in range(B)
    ]
    res = bass_utils.run_bass_kernel_spmd(nc, in_maps, core_ids=list(range(B)))
    return np.stack([res.results[b]["out"] for b in range(B)], axis=0)
